# revision 18
# baseline (speedup 1.0000x reference)
"""GGNN message-passing encoder on 8 Trainium2 NeuronCores.

Data-parallel over batch B=8: core b processes batch element b end-to-end
(its own [N,N] adjacency slice; small GGNN weights replicated), no
collectives. The whole working set (adjT 16.8 MB + state + weights) lives
in SBUF, so the adjacency is read from HBM exactly once and reused for all
NBLOCKS*NSTEPS message-passing matmuls.

Kernel layout: the state is kept feature-major (hT [D=128 part, N=2048
free]) so every matmul has a 512-wide moving operand and runs at full PE
rate in float32r. The adjacency is transposed on-chip (PE transposes,
grouped 4-per-PSUM-bank) into adjT[m, n] once at load time.
"""

import sys

sys.path.insert(0, "/opt/trn_rl_repo")

from contextlib import ExitStack

import numpy as np

import concourse.bass as bass
import concourse.mybir as mybir
import concourse.tile as tile
from concourse import bacc
from concourse.bass import IndirectOffsetOnAxis
from concourse.masks import make_identity

P = 128
B = 8
NL = 1024
LL = 1024
N = NL + LL          # 2048 nodes
D = 128
V = 50000
NBLOCKS = 5
NSTEPS = 3
NT = N // P          # 16 node tiles
CH = 512             # n-chunk (PSUM bank width in fp32)
NCH = N // CH        # 4 chunks
F32 = mybir.dt.float32
F32R = mybir.dt.float32r
I32 = mybir.dt.int32

N_CORES = 8
COPIES_ON_ACT = False


def _r(ap):
    """View an fp32 AP as float32r for full-rate PE matmuls."""
    return ap.bitcast(F32R)


def build_nc(nblocks=NBLOCKS, nsteps=NSTEPS):
    nc = bacc.Bacc("TRN2", target_bir_lowering=False, debug=False,
                   num_devices=N_CORES)

    # ---- per-core DRAM tensors (each core gets its own batch slice) ----
    adj = nc.dram_tensor("adj", [N, N], F32, kind="ExternalInput").ap()
    nidx = nc.dram_tensor("nidx", [NL], I32, kind="ExternalInput").ap()
    lidx = nc.dram_tensor("lidx", [LL], I32, kind="ExternalInput").ap()
    text = nc.dram_tensor("text", [NL], I32, kind="ExternalInput").ap()
    res = nc.dram_tensor("res", [NL], F32, kind="ExternalInput").ap()
    tok_emb = nc.dram_tensor("tok_emb", [V, D - 1], F32,
                             kind="ExternalInput").ap()
    tok_emb1 = nc.dram_tensor("tok_emb1", [V, D], F32,
                              kind="ExternalInput").ap()
    in_W = nc.dram_tensor("in_W", [NBLOCKS, D, D], F32,
                          kind="ExternalInput").ap()
    in_b = nc.dram_tensor("in_b", [NBLOCKS, D], F32, kind="ExternalInput").ap()
    ug_W = nc.dram_tensor("ug_W", [NBLOCKS, 2 * D, D], F32,
                          kind="ExternalInput").ap()
    ug_b = nc.dram_tensor("ug_b", [NBLOCKS, D], F32, kind="ExternalInput").ap()
    rg_W = nc.dram_tensor("rg_W", [NBLOCKS, 2 * D, D], F32,
                          kind="ExternalInput").ap()
    rg_b = nc.dram_tensor("rg_b", [NBLOCKS, D], F32, kind="ExternalInput").ap()
    ht_W = nc.dram_tensor("ht_W", [NBLOCKS, 2 * D, D], F32,
                          kind="ExternalInput").ap()
    ht_b = nc.dram_tensor("ht_b", [NBLOCKS, D], F32, kind="ExternalInput").ap()
    res2_W = nc.dram_tensor("res2_W", [D, 1], F32, kind="ExternalInput").ap()
    res2_b = nc.dram_tensor("res2_b", [1], F32, kind="ExternalInput").ap()

    xout = nc.dram_tensor("xout", [NL, D], F32, kind="ExternalOutput").ap()
    smout = nc.dram_tensor("smout", [1, NL], F32, kind="ExternalOutput").ap()
    lossout = nc.dram_tensor("lossout", [1, 1], F32,
                             kind="ExternalOutput").ap()

    with tile.TileContext(nc) as tc, ExitStack() as ctx:
        p_adjT = ctx.enter_context(tc.tile_pool(name="adjT", bufs=1))
        p_stage = ctx.enter_context(tc.tile_pool(name="stage", bufs=2))
        p_state = ctx.enter_context(tc.tile_pool(name="state", bufs=1))
        p_ch = ctx.enter_context(tc.tile_pool(name="ch", bufs=2))
        p_msg = ctx.enter_context(tc.tile_pool(name="msg", bufs=3))
        p_w = ctx.enter_context(tc.tile_pool(name="w", bufs=2))
        p_const = ctx.enter_context(tc.tile_pool(name="const", bufs=1))
        p_row = ctx.enter_context(tc.tile_pool(name="row", bufs=3))
        ps_tp = ctx.enter_context(
            tc.tile_pool(name="ps_tp", bufs=3, space="PSUM"))
        ps_mm = ctx.enter_context(
            tc.tile_pool(name="ps_mm", bufs=2, space="PSUM"))
        ps_g = ctx.enter_context(
            tc.tile_pool(name="ps_g", bufs=3, space="PSUM"))

        ident = p_const.tile([P, P], F32)
        make_identity(nc, ident[:])

        # persistent state, feature-major: hT[d, n]
        hT = p_state.tile([P, N], F32)
        # adjT_big[p, mt*N + n] = adj[n, mt*128 + p]
        adjT_big = p_adjT.tile([P, NT * N], F32)
        adjT_v = adjT_big.rearrange("p (m n) -> p m n", m=NT)

        # ---- biases / small constants ----
        bias_in = p_const.tile([P, NBLOCKS], F32)
        nc.sync.dma_start(bias_in[:], in_b.rearrange("b d -> d b"))
        bias_ug = p_const.tile([P, NBLOCKS], F32)
        nc.sync.dma_start(bias_ug[:], ug_b.rearrange("b d -> d b"))
        bias_rg = p_const.tile([P, NBLOCKS], F32)
        nc.sync.dma_start(bias_rg[:], rg_b.rearrange("b d -> d b"))
        bias_ht = p_const.tile([P, NBLOCKS], F32)
        nc.sync.dma_start(bias_ht[:], ht_b.rearrange("b d -> d b"))
        res2w_s = p_const.tile([P, 1], F32)
        nc.sync.dma_start(res2w_s[:], res2_W[:, :])
        res2w = p_const.tile([P, 1], F32)
        nc.vector.tensor_copy(_r(res2w[:]), res2w_s[:])
        res2b = p_const.tile([1, 1], F32)
        nc.sync.dma_start(res2b[:], res2_b.rearrange("(a b) -> a b", a=1))
        res_row = p_const.tile([1, NL], F32)
        nc.sync.dma_start(res_row[:], res.rearrange("(a n) -> a n", a=1))

        # gather indices, one per partition: idx_t[p, t] = idx[t*128 + p]
        nidx_t = p_const.tile([P, NL // P], I32)
        nc.sync.dma_start(nidx_t[:], nidx.rearrange("(t p) -> p t", p=P))
        lidx_t = p_const.tile([P, LL // P], I32)
        nc.sync.dma_start(lidx_t[:], lidx.rearrange("(t p) -> p t", p=P))
        text_t = p_const.tile([P, NL // P], I32)
        nc.sync.dma_start(text_t[:], text.rearrange("(t p) -> p t", p=P))

        # ---- adjacency load + on-chip transpose ----
        # staging tile holds adj[nb*128:(nb+1)*128, half*1024:(half+1)*1024]
        for nb in range(NT):
            for q in range(4):
                st = p_stage.tile([P, CH], F32, tag="adj")
                nc.sync.dma_start(
                    st[:], adj[nb * P:(nb + 1) * P, q * CH:(q + 1) * CH])
                mt0 = q * 4
                pt = ps_tp.tile([P, CH], F32, tag="tp")
                for j in range(4):
                    nc.tensor.transpose(pt[:, j * P:(j + 1) * P],
                                        st[:, j * P:(j + 1) * P], ident[:])
                # strided scatter into adjT_big: 4 m-tiles, n-block nb
                nc.vector.tensor_copy(
                    _r(adjT_v[:, mt0:mt0 + 4, nb * P:(nb + 1) * P]),
                    pt[:].rearrange("p (m n) -> p m n", m=4))

        # ---- embeddings -> hT (initial x, feature-major) ----
        # node embedding tile = [tok_emb row, text scalar] (128 features),
        # assembled node-major then PE-transposed into hT
        for g in range(NL // P // 4):
            pt = ps_tp.tile([P, CH], F32, tag="tp")
            for j in range(4):
                t = g * 4 + j
                ge = p_stage.tile([P, D], F32, tag="gather")
                nc.gpsimd.indirect_dma_start(
                    out=ge[:, 0:D - 1], out_offset=None, in_=tok_emb[:, :],
                    in_offset=IndirectOffsetOnAxis(ap=nidx_t[:, t:t + 1],
                                                   axis=0))
                nc.vector.tensor_copy(ge[:, D - 1:D], text_t[:, t:t + 1])
                nc.tensor.transpose(pt[:, j * P:(j + 1) * P], ge[:],
                                    ident[:])
            nc.vector.tensor_copy(_r(hT[:, g * CH:(g + 1) * CH]), pt[:])

        for g in range(LL // P // 4):
            pt = ps_tp.tile([P, CH], F32, tag="tp")
            for j in range(4):
                t = g * 4 + j
                ge1 = p_stage.tile([P, D], F32, tag="gather")
                nc.gpsimd.indirect_dma_start(
                    out=ge1[:], out_offset=None, in_=tok_emb1[:, :],
                    in_offset=IndirectOffsetOnAxis(ap=lidx_t[:, t:t + 1],
                                                   axis=0))
                nc.tensor.transpose(pt[:, j * P:(j + 1) * P], ge1[:],
                                    ident[:])
            nc.vector.tensor_copy(_r(hT[:, NL + g * CH:NL + (g + 1) * CH]),
                                  pt[:])

        # ---- GGNN blocks ----
        # h_nat is double-buffered; its transpose groups are software-
        # pipelined: group c for the NEXT step is emitted right after this
        # step's chunk-c state update (lagged two chunks on PE), and the
        # final group is carried into the next step's first message-matmul
        # accumulation so PE never sits on the ACT/DVE update roundtrip.
        def new_hnat():
            h_nat = p_state.tile([P, N], F32, tag="h_nat", bufs=2,
                                 name="h_nat")
            return h_nat

        def make_tp(h_dst, c):
            def emit():
                pt = ps_tp.tile([P, CH], F32, tag="tp", name="pt_tp")
                for j in range(4):
                    nb = c * 4 + j
                    nc.tensor.transpose(pt[:, j * P:(j + 1) * P],
                                        hT[:, nb * P:(nb + 1) * P],
                                        ident[:])
                if COPIES_ON_ACT:
                    nc.scalar.copy(_r(h_dst[:, c * CH:(c + 1) * CH]), pt[:])
                else:
                    nc.vector.tensor_copy(_r(h_dst[:, c * CH:(c + 1) * CH]),
                                          pt[:])
            return emit

        carry_tp = None
        for blk in range(nblocks):
            ws = p_stage.tile([P, 2, D], F32, tag="wstage")
            nc.sync.dma_start(ws[:, 0, :], in_W[blk])
            w_in = p_w.tile([P, D], F32, tag="w_in")
            nc.vector.tensor_copy(_r(w_in[:]), ws[:, 0, :])
            ws = p_stage.tile([P, 2, D], F32, tag="wstage")
            nc.sync.dma_start(ws[:],
                              ug_W[blk].rearrange("(k p) d -> p k d", p=P))
            w_ug = p_w.tile([P, 2, D], F32, tag="w_ug")
            nc.vector.tensor_copy(_r(w_ug[:]), ws[:])
            ws = p_stage.tile([P, 2, D], F32, tag="wstage")
            nc.sync.dma_start(ws[:],
                              rg_W[blk].rearrange("(k p) d -> p k d", p=P))
            w_rg = p_w.tile([P, 2, D], F32, tag="w_rg")
            nc.vector.tensor_copy(_r(w_rg[:]), ws[:])
            ws = p_stage.tile([P, 2, D], F32, tag="wstage")
            nc.sync.dma_start(ws[:],
                              ht_W[blk].rearrange("(k p) d -> p k d", p=P))
            w_ht = p_w.tile([P, 2, D], F32, tag="w_ht")
            nc.vector.tensor_copy(_r(w_ht[:]), ws[:])

            # h = x @ in_W + in_b chunk-wise in place on hT, with the
            # transpose groups for step 0 pipelined one chunk behind
            h_cur = new_hnat()
            tp_q = []
            for c in range(NCH):
                pm = ps_g.tile([P, CH], F32, tag="g")
                nc.tensor.matmul(pm[:], _r(w_in[:]),
                                 _r(hT[:, c * CH:(c + 1) * CH]),
                                 start=True, stop=True)
                nc.vector.tensor_scalar(
                    out=_r(hT[:, c * CH:(c + 1) * CH]), in0=pm[:],
                    scalar1=bias_in[:, blk:blk + 1], scalar2=None,
                    op0=mybir.AluOpType.add)
                if tp_q:
                    tp_q.pop(0)()
                tp_q.append(make_tp(h_cur, c))
            carry_tp = tp_q.pop(0)

            for step in range(nsteps):
                last_step = step == nsteps - 1
                h_next = None if last_step else new_hnat()

                tails = []  # pending ht-candidate finishers, lag 1
                tps = []    # pending transpose groups for h_next, lag 2
                for c in range(NCH):
                    cs = slice(c * CH, (c + 1) * CH)
                    pmsg = ps_mm.tile([P, CH], F32, tag="m", name="pmsg")
                    pz = ps_g.tile([P, CH], F32, tag="g", name="pz")
                    pr = ps_g.tile([P, CH], F32, tag="g", name="pr")

                    # msgs first half
                    for mt in range(8):
                        nc.tensor.matmul(
                            pmsg[:], _r(h_cur[:, mt * P:(mt + 1) * P]),
                            _r(adjT_v[:, mt, c * CH:(c + 1) * CH]),
                            start=(mt == 0), stop=False)
                    # gate h-halves: no msgs dependency, keeps PE busy and
                    # shortens the post-copy critical path
                    nc.tensor.matmul(pz[:], _r(w_ug[:, 0, :]), _r(hT[:, cs]),
                                     start=True, stop=False)
                    nc.tensor.matmul(pr[:], _r(w_rg[:, 0, :]), _r(hT[:, cs]),
                                     start=True, stop=False)
                    if tails:
                        tails[0][0]()  # t_{c-1} rh-half
                    # msgs second half, with the carried transpose group
                    # (writes h_cur chunk 3 = m-tiles 12..15) at mt == 8
                    for mt in range(8, NT):
                        if mt == 8 and c == 0 and carry_tp is not None:
                            carry_tp()
                            carry_tp = None
                        nc.tensor.matmul(
                            pmsg[:], _r(h_cur[:, mt * P:(mt + 1) * P]),
                            _r(adjT_v[:, mt, c * CH:(c + 1) * CH]),
                            start=False, stop=(mt == NT - 1))
                    msgs_c = p_msg.tile([P, CH], F32, tag="msg",
                                        name="msgs_c")
                    nc.vector.tensor_copy(_r(msgs_c[:]), pmsg[:])

                    if tails:
                        tails.pop(0)[1]()  # t_{c-1} finish
                    if c >= 2 and tps:
                        tps.pop(0)()

                    # gate msgs-halves + activations
                    nc.tensor.matmul(pz[:], _r(w_ug[:, 1, :]), _r(msgs_c[:]),
                                     start=False, stop=True)
                    z_sb = p_ch.tile([P, CH], F32, tag="z")
                    nc.scalar.activation(z_sb[:], pz[:],
                                         mybir.ActivationFunctionType.Sigmoid,
                                         bias=bias_ug[:, blk:blk + 1])
                    nc.tensor.matmul(pr[:], _r(w_rg[:, 1, :]), _r(msgs_c[:]),
                                     start=False, stop=True)
                    r_c = p_ch.tile([P, CH], F32, tag="r")
                    nc.scalar.activation(r_c[:], pr[:],
                                         mybir.ActivationFunctionType.Sigmoid,
                                         bias=bias_rg[:, blk:blk + 1])
                    rh_c = p_ch.tile([P, CH], F32, tag="rh")
                    nc.vector.tensor_mul(_r(rh_c[:]), r_c[:], hT[:, cs])

                    # build this chunk's tail (two phases)
                    def make_tail(c, z_sb, msgs_c, rh_c):
                        cs = slice(c * CH, (c + 1) * CH)
                        pt2 = ps_g.tile([P, CH], F32, tag="g", name="pt2")

                        def phase_a():
                            nc.tensor.matmul(pt2[:], _r(w_ht[:, 0, :]),
                                             _r(rh_c[:]), start=True,
                                             stop=False)

                        def phase_b(msgs_c=msgs_c):
                            nc.tensor.matmul(pt2[:], _r(w_ht[:, 1, :]),
                                             _r(msgs_c[:]),
                                             start=False, stop=True)
                            t_sb = p_ch.tile([P, CH], F32, tag="t",
                                             name="t_sb")
                            nc.scalar.activation(
                                t_sb[:], pt2[:],
                                mybir.ActivationFunctionType.Tanh,
                                bias=bias_ht[:, blk:blk + 1])
                            d_c = p_ch.tile([P, CH], F32, tag="d", bufs=1,
                                            name="d_c")
                            nc.vector.tensor_sub(d_c[:], hT[:, cs], t_sb[:])
                            nc.vector.tensor_mul(d_c[:], z_sb[:], d_c[:])
                            nc.vector.tensor_add(_r(hT[:, cs]), d_c[:],
                                                 t_sb[:])
                        return phase_a, phase_b

                    tails.append(make_tail(c, z_sb, msgs_c, rh_c))
                    if h_next is not None:
                        tps.append(make_tp(h_next, c))

                # epilogue: cover the rh_3 roundtrip with tp(2), then finish
                if tps:
                    tps.pop(0)()
                pa, pb = tails.pop(0)
                pa()
                pb()
                carry_tp = tps.pop(0) if tps else None
                h_cur = h_next

        # ---- outputs ----
        # x output: node half of hT back to node-major
        xout_v = xout.rearrange("(a p) d -> p a d", p=P)
        for g in range(2):
            pt = ps_tp.tile([P, CH], F32, tag="tp")
            for j in range(4):
                nb = g * 4 + j
                nc.tensor.transpose(pt[:, j * P:(j + 1) * P],
                                    hT[:, nb * P:(nb + 1) * P], ident[:])
            xs = p_stage.tile([P, CH], F32, tag="adj")
            nc.vector.tensor_copy(xs[:], pt[:])
            nc.sync.dma_start(xout_v[:, g * 4:(g + 1) * 4, :],
                              xs[:].rearrange("p (a d) -> p a d", a=4))

        # logits = x[:, :NL] @ res2_W + res2_b
        logits = p_row.tile([1, NL], F32, tag="row")
        for c in range(NL // CH):
            pl = ps_g.tile([1, CH], F32, tag="g")
            nc.tensor.matmul(pl[:], _r(res2w[:]),
                             _r(hT[:, c * CH:(c + 1) * CH]),
                             start=True, stop=True)
            nc.scalar.activation(logits[:, c * CH:(c + 1) * CH], pl[:],
                                 mybir.ActivationFunctionType.Identity,
                                 bias=res2b[:])

        # resmask = (input_node == 2); logits = where(mask, logits, -1e9)
        nidx_row = p_row.tile([1, NL], I32, tag="row")
        nc.sync.dma_start(nidx_row[:], nidx.rearrange("(a n) -> a n", a=1))
        mask = p_row.tile([1, NL], I32, tag="row")
        nc.vector.tensor_scalar(out=mask[:], in0=nidx_row[:], scalar1=2,
                                scalar2=None, op0=mybir.AluOpType.is_equal)
        masked = p_row.tile([1, NL], F32, tag="row")
        nc.vector.memset(masked[:], -1e9)
        nc.vector.copy_predicated(masked[:], mask[:], logits[:])

        # softmax along the row
        mx = p_const.tile([1, 1], F32)
        nc.vector.reduce_max(mx[:], masked[:], axis=mybir.AxisListType.X,
                             negate=True)
        ex = p_row.tile([1, NL], F32, tag="row")
        nc.scalar.activation(ex[:], masked[:],
                             mybir.ActivationFunctionType.Exp,
                             bias=mx[:], scale=1.0)
        sm_sum = p_const.tile([1, 1], F32)
        nc.vector.reduce_sum(sm_sum[:], ex[:], axis=mybir.AxisListType.X)
        nc.vector.reciprocal(sm_sum[:], sm_sum[:])
        smx = p_row.tile([1, NL], F32, tag="row")
        nc.vector.tensor_scalar_mul(smx[:], ex[:], sm_sum[:])
        nc.sync.dma_start(smout[:, :], smx[:])

        # loss = -(log(clip(softmax, 1e-10, 1)) * res).sum()
        cl = p_row.tile([1, NL], F32, tag="row")
        nc.vector.tensor_scalar(out=cl[:], in0=smx[:], scalar1=1e-10,
                                scalar2=1.0, op0=mybir.AluOpType.max,
                                op1=mybir.AluOpType.min)
        lnr = p_row.tile([1, NL], F32, tag="row")
        nc.scalar.activation(lnr[:], cl[:], mybir.ActivationFunctionType.Ln)
        prd = p_row.tile([1, NL], F32, tag="row")
        nc.vector.tensor_mul(prd[:], lnr[:], res_row[:])
        ls = p_const.tile([1, 1], F32)
        nc.vector.reduce_sum(ls[:], prd[:], axis=mybir.AxisListType.X)
        nc.vector.tensor_scalar_mul(ls[:], ls[:], -1.0)
        nc.sync.dma_start(lossout[:, :], ls[:])

    nc.compile()
    return nc


_NC = None
LAST_RESULT = None


def _get_nc():
    global _NC
    if _NC is None:
        _NC = build_nc()
    return _NC


def kernel(**inputs):
    nc = _get_nc()

    adj = np.ascontiguousarray(np.asarray(inputs["inputad"], np.float32))
    nidx = np.asarray(inputs["input_node"]).astype(np.int32)
    lidx = np.asarray(inputs["linenode"]).astype(np.int32)
    text = np.asarray(inputs["inputtext"]).astype(np.int32)
    res = np.asarray(inputs["res"]).astype(np.float32)
    shared = {
        "tok_emb": np.ascontiguousarray(np.asarray(inputs["tok_emb"], np.float32)),
        "tok_emb1": np.ascontiguousarray(np.asarray(inputs["tok_emb1"], np.float32)),
        "in_W": np.ascontiguousarray(np.asarray(inputs["in_W"], np.float32)),
        "in_b": np.ascontiguousarray(np.asarray(inputs["in_b"], np.float32)),
        "ug_W": np.ascontiguousarray(np.asarray(inputs["ug_W"], np.float32)),
        "ug_b": np.ascontiguousarray(np.asarray(inputs["ug_b"], np.float32)),
        "rg_W": np.ascontiguousarray(np.asarray(inputs["rg_W"], np.float32)),
        "rg_b": np.ascontiguousarray(np.asarray(inputs["rg_b"], np.float32)),
        "ht_W": np.ascontiguousarray(np.asarray(inputs["ht_W"], np.float32)),
        "ht_b": np.ascontiguousarray(np.asarray(inputs["ht_b"], np.float32)),
        "res2_W": np.ascontiguousarray(np.asarray(inputs["res2_W"], np.float32)),
        "res2_b": np.ascontiguousarray(np.asarray(inputs["res2_b"], np.float32)),
    }
    in_maps = []
    for b in range(N_CORES):
        in_maps.append({
            "adj": np.ascontiguousarray(adj[b]),
            "nidx": np.ascontiguousarray(nidx[b]),
            "lidx": np.ascontiguousarray(lidx[b]),
            "text": np.ascontiguousarray(text[b]),
            "res": np.ascontiguousarray(res[b]),
            **shared,
        })

    from concourse.bass_utils import run_bass_kernel_spmd
    global LAST_RESULT
    LAST_RESULT = run_bass_kernel_spmd(nc, in_maps,
                                       core_ids=list(range(N_CORES)))

    loss = np.zeros([B], np.float32)
    softmax = np.zeros([B, NL], np.float32)
    x = np.zeros([B, NL, D], np.float32)
    for b in range(N_CORES):
        r = LAST_RESULT.results[b]
        loss[b] = r["lossout"][0, 0]
        softmax[b] = r["smout"][0]
        x[b] = r["xout"]
    return loss, softmax, x


# revision 19
# speedup vs baseline: 1.1274x; 1.1274x over previous
"""GGNN message-passing encoder on 8 Trainium2 NeuronCores.

Data-parallel over batch B=8: core b processes batch element b end-to-end
(its own [N,N] adjacency slice; small GGNN weights replicated), no
collectives. The whole working set (adjT 16.8 MB + state + weights) lives
in SBUF, so the adjacency is read from HBM exactly once and reused for all
NBLOCKS*NSTEPS message-passing matmuls.

Kernel layout: the state is kept feature-major (hT [D=128 part, N=2048
free]) so every matmul has a 512-wide moving operand and runs at full PE
rate in float32r. The adjacency is transposed on-chip (PE transposes,
grouped 4-per-PSUM-bank) into adjT[m, n] once at load time.
"""

import sys

sys.path.insert(0, "/opt/trn_rl_repo")

from contextlib import ExitStack

import numpy as np

import concourse.bass as bass
import concourse.mybir as mybir
import concourse.tile as tile
from concourse import bacc
from concourse.bass import IndirectOffsetOnAxis
from concourse.masks import make_identity

P = 128
B = 8
NL = 1024
LL = 1024
N = NL + LL          # 2048 nodes
D = 128
V = 50000
NBLOCKS = 5
NSTEPS = 3
NT = N // P          # 16 node tiles
CH = 512             # n-chunk (PSUM bank width in fp32)
NCH = N // CH        # 4 chunks
F32 = mybir.dt.float32
F32R = mybir.dt.float32r
I32 = mybir.dt.int32

N_CORES = 8
COPIES_ON_ACT = False


def _r(ap):
    """View an fp32 AP as float32r for full-rate PE matmuls."""
    return ap.bitcast(F32R)


def build_nc(nblocks=NBLOCKS, nsteps=NSTEPS):
    nc = bacc.Bacc("TRN2", target_bir_lowering=False, debug=False,
                   num_devices=N_CORES)

    # ---- per-core DRAM tensors (each core gets its own batch slice) ----
    adj = nc.dram_tensor("adj", [N, N], F32, kind="ExternalInput").ap()
    nidx = nc.dram_tensor("nidx", [NL], I32, kind="ExternalInput").ap()
    lidx = nc.dram_tensor("lidx", [LL], I32, kind="ExternalInput").ap()
    text = nc.dram_tensor("text", [NL], I32, kind="ExternalInput").ap()
    res = nc.dram_tensor("res", [NL], F32, kind="ExternalInput").ap()
    tok_emb = nc.dram_tensor("tok_emb", [V, D - 1], F32,
                             kind="ExternalInput").ap()
    tok_emb1 = nc.dram_tensor("tok_emb1", [V, D], F32,
                              kind="ExternalInput").ap()
    in_W = nc.dram_tensor("in_W", [NBLOCKS, D, D], F32,
                          kind="ExternalInput").ap()
    in_b = nc.dram_tensor("in_b", [NBLOCKS, D], F32, kind="ExternalInput").ap()
    ug_W = nc.dram_tensor("ug_W", [NBLOCKS, 2 * D, D], F32,
                          kind="ExternalInput").ap()
    ug_b = nc.dram_tensor("ug_b", [NBLOCKS, D], F32, kind="ExternalInput").ap()
    rg_W = nc.dram_tensor("rg_W", [NBLOCKS, 2 * D, D], F32,
                          kind="ExternalInput").ap()
    rg_b = nc.dram_tensor("rg_b", [NBLOCKS, D], F32, kind="ExternalInput").ap()
    ht_W = nc.dram_tensor("ht_W", [NBLOCKS, 2 * D, D], F32,
                          kind="ExternalInput").ap()
    ht_b = nc.dram_tensor("ht_b", [NBLOCKS, D], F32, kind="ExternalInput").ap()
    res2_W = nc.dram_tensor("res2_W", [D, 1], F32, kind="ExternalInput").ap()
    res2_b = nc.dram_tensor("res2_b", [1], F32, kind="ExternalInput").ap()

    xout = nc.dram_tensor("xout", [NL, D], F32, kind="ExternalOutput").ap()
    smout = nc.dram_tensor("smout", [1, NL], F32, kind="ExternalOutput").ap()
    lossout = nc.dram_tensor("lossout", [1, 1], F32,
                             kind="ExternalOutput").ap()

    with tile.TileContext(nc) as tc, ExitStack() as ctx:
        p_adjT = ctx.enter_context(tc.tile_pool(name="adjT", bufs=1))
        p_state = ctx.enter_context(tc.tile_pool(name="state", bufs=1))
        p_ch = ctx.enter_context(tc.tile_pool(name="ch", bufs=2))
        p_msg = ctx.enter_context(tc.tile_pool(name="msg", bufs=3))
        p_w = ctx.enter_context(tc.tile_pool(name="w", bufs=2))
        p_const = ctx.enter_context(tc.tile_pool(name="const", bufs=1))
        ps_tp = ctx.enter_context(
            tc.tile_pool(name="ps_tp", bufs=3, space="PSUM"))
        ps_mm = ctx.enter_context(
            tc.tile_pool(name="ps_mm", bufs=2, space="PSUM"))
        ps_g = ctx.enter_context(
            tc.tile_pool(name="ps_g", bufs=3, space="PSUM"))

        ident = p_const.tile([P, P], F32)
        make_identity(nc, ident[:])

        # persistent state, feature-major: hT[d, n]
        hT = p_state.tile([P, N], F32)
        # adjT_big[p, mt*N + n] = adj[n, mt*128 + p]
        adjT_big = p_adjT.tile([P, NT * N], F32)
        adjT_v = adjT_big.rearrange("p (m n) -> p m n", m=NT)

        # ---- biases / small constants ----
        bias_in = p_const.tile([P, NBLOCKS], F32)
        nc.sync.dma_start(bias_in[:], in_b.rearrange("b d -> d b"))
        bias_ug = p_const.tile([P, NBLOCKS], F32)
        nc.sync.dma_start(bias_ug[:], ug_b.rearrange("b d -> d b"))
        bias_rg = p_const.tile([P, NBLOCKS], F32)
        nc.sync.dma_start(bias_rg[:], rg_b.rearrange("b d -> d b"))
        bias_ht = p_const.tile([P, NBLOCKS], F32)
        nc.sync.dma_start(bias_ht[:], ht_b.rearrange("b d -> d b"))
        res2w_s = p_const.tile([P, 1], F32)
        nc.sync.dma_start(res2w_s[:], res2_W[:, :])
        res2w = p_const.tile([P, 1], F32)
        nc.vector.tensor_copy(_r(res2w[:]), res2w_s[:])
        res2b = p_const.tile([1, 1], F32)
        nc.sync.dma_start(res2b[:], res2_b.rearrange("(a b) -> a b", a=1))
        res_row = p_const.tile([1, NL], F32)
        nc.sync.dma_start(res_row[:], res.rearrange("(a n) -> a n", a=1))

        # gather indices, one per partition: idx_t[p, t] = idx[t*128 + p]
        nidx_t = p_const.tile([P, NL // P], I32)
        nc.sync.dma_start(nidx_t[:], nidx.rearrange("(t p) -> p t", p=P))
        lidx_t = p_const.tile([P, LL // P], I32)
        nc.sync.dma_start(lidx_t[:], lidx.rearrange("(t p) -> p t", p=P))
        text_t = p_const.tile([P, NL // P], I32)
        nc.sync.dma_start(text_t[:], text.rearrange("(t p) -> p t", p=P))

        # ---- adjacency load + on-chip transpose ----
        # full-row staging: 8KB contiguous per partition line keeps the
        # HBM DMA near peak rate (2KB lines measured at only ~107 GB/s)
        with tc.tile_pool(name="stage", bufs=2) as p_stage:
            for nb in range(NT):
                st = p_stage.tile([P, N], F32, tag="adj")
                nc.sync.dma_start(st[:], adj[nb * P:(nb + 1) * P, :])
                for q in range(4):
                    mt0 = q * 4
                    pt = ps_tp.tile([P, CH], F32, tag="tp")
                    for j in range(4):
                        nc.tensor.transpose(
                            pt[:, j * P:(j + 1) * P],
                            st[:, (q * 4 + j) * P:(q * 4 + j + 1) * P],
                            ident[:])
                    # strided scatter into adjT_big: 4 m-tiles, n-block nb
                    nc.vector.tensor_copy(
                        _r(adjT_v[:, mt0:mt0 + 4, nb * P:(nb + 1) * P]),
                        pt[:].rearrange("p (m n) -> p m n", m=4))
        p_row = ctx.enter_context(tc.tile_pool(name="row", bufs=3))

        # ---- embeddings -> hT (initial x, feature-major) ----
        # node embedding tile = [tok_emb row, text scalar] (128 features),
        # assembled node-major then PE-transposed into hT
        for g in range(NL // P // 4):
            pt = ps_tp.tile([P, CH], F32, tag="tp")
            for j in range(4):
                t = g * 4 + j
                ge = p_ch.tile([P, D], F32, tag="z")
                nc.gpsimd.indirect_dma_start(
                    out=ge[:, 0:D - 1], out_offset=None, in_=tok_emb[:, :],
                    in_offset=IndirectOffsetOnAxis(ap=nidx_t[:, t:t + 1],
                                                   axis=0))
                nc.vector.tensor_copy(ge[:, D - 1:D], text_t[:, t:t + 1])
                nc.tensor.transpose(pt[:, j * P:(j + 1) * P], ge[:],
                                    ident[:])
            nc.vector.tensor_copy(_r(hT[:, g * CH:(g + 1) * CH]), pt[:])

        for g in range(LL // P // 4):
            pt = ps_tp.tile([P, CH], F32, tag="tp")
            for j in range(4):
                t = g * 4 + j
                ge1 = p_ch.tile([P, D], F32, tag="z")
                nc.gpsimd.indirect_dma_start(
                    out=ge1[:], out_offset=None, in_=tok_emb1[:, :],
                    in_offset=IndirectOffsetOnAxis(ap=lidx_t[:, t:t + 1],
                                                   axis=0))
                nc.tensor.transpose(pt[:, j * P:(j + 1) * P], ge1[:],
                                    ident[:])
            nc.vector.tensor_copy(_r(hT[:, NL + g * CH:NL + (g + 1) * CH]),
                                  pt[:])

        # ---- GGNN blocks ----
        # h_nat is double-buffered; its transpose groups are software-
        # pipelined: group c for the NEXT step is emitted right after this
        # step's chunk-c state update (lagged two chunks on PE), and the
        # final group is carried into the next step's first message-matmul
        # accumulation so PE never sits on the ACT/DVE update roundtrip.
        def new_hnat():
            h_nat = p_state.tile([P, N], F32, tag="h_nat", bufs=2,
                                 name="h_nat")
            return h_nat

        def make_tp(h_dst, c):
            def emit():
                pt = ps_tp.tile([P, CH], F32, tag="tp", name="pt_tp")
                for j in range(4):
                    nb = c * 4 + j
                    nc.tensor.transpose(pt[:, j * P:(j + 1) * P],
                                        hT[:, nb * P:(nb + 1) * P],
                                        ident[:])
                if COPIES_ON_ACT:
                    nc.scalar.copy(_r(h_dst[:, c * CH:(c + 1) * CH]), pt[:])
                else:
                    nc.vector.tensor_copy(_r(h_dst[:, c * CH:(c + 1) * CH]),
                                          pt[:])
            return emit

        carry_tp = None
        for blk in range(nblocks):
            ws = p_ch.tile([P, 2, D], F32, tag="t")
            nc.sync.dma_start(ws[:, 0, :], in_W[blk])
            w_in = p_w.tile([P, D], F32, tag="w_in")
            nc.vector.tensor_copy(_r(w_in[:]), ws[:, 0, :])
            ws = p_ch.tile([P, 2, D], F32, tag="t")
            nc.sync.dma_start(ws[:],
                              ug_W[blk].rearrange("(k p) d -> p k d", p=P))
            w_ug = p_w.tile([P, 2, D], F32, tag="w_ug")
            nc.vector.tensor_copy(_r(w_ug[:]), ws[:])
            ws = p_ch.tile([P, 2, D], F32, tag="t")
            nc.sync.dma_start(ws[:],
                              rg_W[blk].rearrange("(k p) d -> p k d", p=P))
            w_rg = p_w.tile([P, 2, D], F32, tag="w_rg")
            nc.vector.tensor_copy(_r(w_rg[:]), ws[:])
            ws = p_ch.tile([P, 2, D], F32, tag="t")
            nc.sync.dma_start(ws[:],
                              ht_W[blk].rearrange("(k p) d -> p k d", p=P))
            w_ht = p_w.tile([P, 2, D], F32, tag="w_ht")
            nc.vector.tensor_copy(_r(w_ht[:]), ws[:])

            # h = x @ in_W + in_b chunk-wise in place on hT, with the
            # transpose groups for step 0 pipelined one chunk behind
            h_cur = new_hnat()
            tp_q = []
            for c in range(NCH):
                pm = ps_g.tile([P, CH], F32, tag="g")
                nc.tensor.matmul(pm[:], _r(w_in[:]),
                                 _r(hT[:, c * CH:(c + 1) * CH]),
                                 start=True, stop=True)
                nc.vector.tensor_scalar(
                    out=_r(hT[:, c * CH:(c + 1) * CH]), in0=pm[:],
                    scalar1=bias_in[:, blk:blk + 1], scalar2=None,
                    op0=mybir.AluOpType.add)
                if tp_q:
                    tp_q.pop(0)()
                tp_q.append(make_tp(h_cur, c))
            carry_tp = tp_q.pop(0)

            for step in range(nsteps):
                last_step = step == nsteps - 1
                h_next = None if last_step else new_hnat()

                tails = []  # pending ht-candidate finishers, lag 1
                tps = []    # pending transpose groups for h_next, lag 2
                for c in range(NCH):
                    cs = slice(c * CH, (c + 1) * CH)
                    pmsg = ps_mm.tile([P, CH], F32, tag="m", name="pmsg")
                    pz = ps_g.tile([P, CH], F32, tag="g", name="pz")
                    pr = ps_g.tile([P, CH], F32, tag="g", name="pr")

                    # msgs first half
                    for mt in range(8):
                        nc.tensor.matmul(
                            pmsg[:], _r(h_cur[:, mt * P:(mt + 1) * P]),
                            _r(adjT_v[:, mt, c * CH:(c + 1) * CH]),
                            start=(mt == 0), stop=False)
                    # gate h-halves: no msgs dependency, keeps PE busy and
                    # shortens the post-copy critical path
                    nc.tensor.matmul(pz[:], _r(w_ug[:, 0, :]), _r(hT[:, cs]),
                                     start=True, stop=False)
                    nc.tensor.matmul(pr[:], _r(w_rg[:, 0, :]), _r(hT[:, cs]),
                                     start=True, stop=False)
                    if tails:
                        tails[0][0]()  # t_{c-1} rh-half
                    # msgs second half, with the carried transpose group
                    # (writes h_cur chunk 3 = m-tiles 12..15) at mt == 8
                    for mt in range(8, NT):
                        if mt == 8 and c == 0 and carry_tp is not None:
                            carry_tp()
                            carry_tp = None
                        nc.tensor.matmul(
                            pmsg[:], _r(h_cur[:, mt * P:(mt + 1) * P]),
                            _r(adjT_v[:, mt, c * CH:(c + 1) * CH]),
                            start=False, stop=(mt == NT - 1))
                    msgs_c = p_msg.tile([P, CH], F32, tag="msg",
                                        name="msgs_c")
                    nc.vector.tensor_copy(_r(msgs_c[:]), pmsg[:])

                    if tails:
                        tails.pop(0)[1]()  # t_{c-1} finish
                    if c >= 2 and tps:
                        tps.pop(0)()

                    # gate msgs-halves + activations
                    nc.tensor.matmul(pz[:], _r(w_ug[:, 1, :]), _r(msgs_c[:]),
                                     start=False, stop=True)
                    z_sb = p_ch.tile([P, CH], F32, tag="z")
                    nc.scalar.activation(z_sb[:], pz[:],
                                         mybir.ActivationFunctionType.Sigmoid,
                                         bias=bias_ug[:, blk:blk + 1])
                    nc.tensor.matmul(pr[:], _r(w_rg[:, 1, :]), _r(msgs_c[:]),
                                     start=False, stop=True)
                    r_c = p_ch.tile([P, CH], F32, tag="r")
                    nc.scalar.activation(r_c[:], pr[:],
                                         mybir.ActivationFunctionType.Sigmoid,
                                         bias=bias_rg[:, blk:blk + 1])
                    rh_c = p_ch.tile([P, CH], F32, tag="rh")
                    nc.vector.tensor_mul(_r(rh_c[:]), r_c[:], hT[:, cs])

                    # build this chunk's tail (two phases)
                    def make_tail(c, z_sb, msgs_c, rh_c):
                        cs = slice(c * CH, (c + 1) * CH)
                        pt2 = ps_g.tile([P, CH], F32, tag="g", name="pt2")

                        def phase_a():
                            nc.tensor.matmul(pt2[:], _r(w_ht[:, 0, :]),
                                             _r(rh_c[:]), start=True,
                                             stop=False)

                        def phase_b(msgs_c=msgs_c):
                            nc.tensor.matmul(pt2[:], _r(w_ht[:, 1, :]),
                                             _r(msgs_c[:]),
                                             start=False, stop=True)
                            t_sb = p_ch.tile([P, CH], F32, tag="t",
                                             name="t_sb")
                            nc.scalar.activation(
                                t_sb[:], pt2[:],
                                mybir.ActivationFunctionType.Tanh,
                                bias=bias_ht[:, blk:blk + 1])
                            d_c = p_ch.tile([P, CH], F32, tag="d", bufs=1,
                                            name="d_c")
                            nc.vector.tensor_sub(d_c[:], hT[:, cs], t_sb[:])
                            nc.vector.tensor_mul(d_c[:], z_sb[:], d_c[:])
                            nc.vector.tensor_add(_r(hT[:, cs]), d_c[:],
                                                 t_sb[:])
                        return phase_a, phase_b

                    tails.append(make_tail(c, z_sb, msgs_c, rh_c))
                    if h_next is not None:
                        tps.append(make_tp(h_next, c))

                # epilogue: cover the rh_3 roundtrip with tp(2), then finish
                if tps:
                    tps.pop(0)()
                pa, pb = tails.pop(0)
                pa()
                pb()
                carry_tp = tps.pop(0) if tps else None
                h_cur = h_next

        # ---- outputs ----
        # x output: node half of hT back to node-major
        xout_v = xout.rearrange("(a p) d -> p a d", p=P)
        for g in range(2):
            pt = ps_tp.tile([P, CH], F32, tag="tp")
            for j in range(4):
                nb = g * 4 + j
                nc.tensor.transpose(pt[:, j * P:(j + 1) * P],
                                    hT[:, nb * P:(nb + 1) * P], ident[:])
            xs = p_ch.tile([P, CH], F32, tag="z")
            nc.vector.tensor_copy(xs[:], pt[:])
            nc.sync.dma_start(xout_v[:, g * 4:(g + 1) * 4, :],
                              xs[:].rearrange("p (a d) -> p a d", a=4))

        # logits = x[:, :NL] @ res2_W + res2_b
        logits = p_row.tile([1, NL], F32, tag="row")
        for c in range(NL // CH):
            pl = ps_g.tile([1, CH], F32, tag="g")
            nc.tensor.matmul(pl[:], _r(res2w[:]),
                             _r(hT[:, c * CH:(c + 1) * CH]),
                             start=True, stop=True)
            nc.scalar.activation(logits[:, c * CH:(c + 1) * CH], pl[:],
                                 mybir.ActivationFunctionType.Identity,
                                 bias=res2b[:])

        # resmask = (input_node == 2); logits = where(mask, logits, -1e9)
        nidx_row = p_row.tile([1, NL], I32, tag="row")
        nc.sync.dma_start(nidx_row[:], nidx.rearrange("(a n) -> a n", a=1))
        mask = p_row.tile([1, NL], I32, tag="row")
        nc.vector.tensor_scalar(out=mask[:], in0=nidx_row[:], scalar1=2,
                                scalar2=None, op0=mybir.AluOpType.is_equal)
        masked = p_row.tile([1, NL], F32, tag="row")
        nc.vector.memset(masked[:], -1e9)
        nc.vector.copy_predicated(masked[:], mask[:], logits[:])

        # softmax along the row
        mx = p_const.tile([1, 1], F32)
        nc.vector.reduce_max(mx[:], masked[:], axis=mybir.AxisListType.X,
                             negate=True)
        ex = p_row.tile([1, NL], F32, tag="row")
        nc.scalar.activation(ex[:], masked[:],
                             mybir.ActivationFunctionType.Exp,
                             bias=mx[:], scale=1.0)
        sm_sum = p_const.tile([1, 1], F32)
        nc.vector.reduce_sum(sm_sum[:], ex[:], axis=mybir.AxisListType.X)
        nc.vector.reciprocal(sm_sum[:], sm_sum[:])
        smx = p_row.tile([1, NL], F32, tag="row")
        nc.vector.tensor_scalar_mul(smx[:], ex[:], sm_sum[:])
        nc.sync.dma_start(smout[:, :], smx[:])

        # loss = -(log(clip(softmax, 1e-10, 1)) * res).sum()
        cl = p_row.tile([1, NL], F32, tag="row")
        nc.vector.tensor_scalar(out=cl[:], in0=smx[:], scalar1=1e-10,
                                scalar2=1.0, op0=mybir.AluOpType.max,
                                op1=mybir.AluOpType.min)
        lnr = p_row.tile([1, NL], F32, tag="row")
        nc.scalar.activation(lnr[:], cl[:], mybir.ActivationFunctionType.Ln)
        prd = p_row.tile([1, NL], F32, tag="row")
        nc.vector.tensor_mul(prd[:], lnr[:], res_row[:])
        ls = p_const.tile([1, 1], F32)
        nc.vector.reduce_sum(ls[:], prd[:], axis=mybir.AxisListType.X)
        nc.vector.tensor_scalar_mul(ls[:], ls[:], -1.0)
        nc.sync.dma_start(lossout[:, :], ls[:])

    nc.compile()
    return nc


_NC = None
LAST_RESULT = None


def _get_nc():
    global _NC
    if _NC is None:
        _NC = build_nc()
    return _NC


def kernel(**inputs):
    nc = _get_nc()

    adj = np.ascontiguousarray(np.asarray(inputs["inputad"], np.float32))
    nidx = np.asarray(inputs["input_node"]).astype(np.int32)
    lidx = np.asarray(inputs["linenode"]).astype(np.int32)
    text = np.asarray(inputs["inputtext"]).astype(np.int32)
    res = np.asarray(inputs["res"]).astype(np.float32)
    shared = {
        "tok_emb": np.ascontiguousarray(np.asarray(inputs["tok_emb"], np.float32)),
        "tok_emb1": np.ascontiguousarray(np.asarray(inputs["tok_emb1"], np.float32)),
        "in_W": np.ascontiguousarray(np.asarray(inputs["in_W"], np.float32)),
        "in_b": np.ascontiguousarray(np.asarray(inputs["in_b"], np.float32)),
        "ug_W": np.ascontiguousarray(np.asarray(inputs["ug_W"], np.float32)),
        "ug_b": np.ascontiguousarray(np.asarray(inputs["ug_b"], np.float32)),
        "rg_W": np.ascontiguousarray(np.asarray(inputs["rg_W"], np.float32)),
        "rg_b": np.ascontiguousarray(np.asarray(inputs["rg_b"], np.float32)),
        "ht_W": np.ascontiguousarray(np.asarray(inputs["ht_W"], np.float32)),
        "ht_b": np.ascontiguousarray(np.asarray(inputs["ht_b"], np.float32)),
        "res2_W": np.ascontiguousarray(np.asarray(inputs["res2_W"], np.float32)),
        "res2_b": np.ascontiguousarray(np.asarray(inputs["res2_b"], np.float32)),
    }
    in_maps = []
    for b in range(N_CORES):
        in_maps.append({
            "adj": np.ascontiguousarray(adj[b]),
            "nidx": np.ascontiguousarray(nidx[b]),
            "lidx": np.ascontiguousarray(lidx[b]),
            "text": np.ascontiguousarray(text[b]),
            "res": np.ascontiguousarray(res[b]),
            **shared,
        })

    from concourse.bass_utils import run_bass_kernel_spmd
    global LAST_RESULT
    LAST_RESULT = run_bass_kernel_spmd(nc, in_maps,
                                       core_ids=list(range(N_CORES)))

    loss = np.zeros([B], np.float32)
    softmax = np.zeros([B, NL], np.float32)
    x = np.zeros([B, NL, D], np.float32)
    for b in range(N_CORES):
        r = LAST_RESULT.results[b]
        loss[b] = r["lossout"][0, 0]
        softmax[b] = r["smout"][0]
        x[b] = r["xout"]
    return loss, softmax, x


# revision 24
# speedup vs baseline: 1.1747x; 1.0419x over previous
"""GGNN message-passing encoder on 8 Trainium2 NeuronCores.

Data-parallel over batch B=8: core b processes batch element b end-to-end
(its own [N,N] adjacency slice; small GGNN weights replicated), no
collectives. The whole working set (adjT 16.8 MB + state + weights) lives
in SBUF, so the adjacency is read from HBM exactly once and reused for all
NBLOCKS*NSTEPS message-passing matmuls.

Kernel layout: the state is kept feature-major (hT [D=128 part, N=2048
free]) so every matmul has a 512-wide moving operand and runs at full PE
rate in float32r. The adjacency is transposed on-chip (PE transposes,
grouped 4-per-PSUM-bank) into adjT[m, n] once at load time.
"""

import sys

sys.path.insert(0, "/opt/trn_rl_repo")

from contextlib import ExitStack

import numpy as np

import concourse.bass as bass
import concourse.mybir as mybir
import concourse.tile as tile
from concourse import bacc
from concourse.bass import IndirectOffsetOnAxis
from concourse.masks import make_identity

P = 128
B = 8
NL = 1024
LL = 1024
N = NL + LL          # 2048 nodes
D = 128
V = 50000
NBLOCKS = 5
NSTEPS = 3
NT = N // P          # 16 node tiles
CH = 512             # n-chunk (PSUM bank width in fp32)
NCH = N // CH        # 4 chunks
F32 = mybir.dt.float32
F32R = mybir.dt.float32r
I32 = mybir.dt.int32

N_CORES = 8
COPIES_ON_ACT = False


def _r(ap):
    """View an fp32 AP as float32r for full-rate PE matmuls."""
    return ap.bitcast(F32R)


def build_nc(nblocks=NBLOCKS, nsteps=NSTEPS):
    nc = bacc.Bacc("TRN2", target_bir_lowering=False, debug=False,
                   num_devices=N_CORES)

    # ---- per-core DRAM tensors (each core gets its own batch slice) ----
    adj = nc.dram_tensor("adj", [N, N], F32, kind="ExternalInput").ap()
    nidx = nc.dram_tensor("nidx", [NL], I32, kind="ExternalInput").ap()
    lidx = nc.dram_tensor("lidx", [LL], I32, kind="ExternalInput").ap()
    text = nc.dram_tensor("text", [NL], I32, kind="ExternalInput").ap()
    res = nc.dram_tensor("res", [NL], F32, kind="ExternalInput").ap()
    tok_emb = nc.dram_tensor("tok_emb", [V, D - 1], F32,
                             kind="ExternalInput").ap()
    tok_emb1 = nc.dram_tensor("tok_emb1", [V, D], F32,
                              kind="ExternalInput").ap()
    in_W = nc.dram_tensor("in_W", [NBLOCKS, D, D], F32,
                          kind="ExternalInput").ap()
    in_b = nc.dram_tensor("in_b", [NBLOCKS, D], F32, kind="ExternalInput").ap()
    ug_W = nc.dram_tensor("ug_W", [NBLOCKS, 2 * D, D], F32,
                          kind="ExternalInput").ap()
    ug_b = nc.dram_tensor("ug_b", [NBLOCKS, D], F32, kind="ExternalInput").ap()
    rg_W = nc.dram_tensor("rg_W", [NBLOCKS, 2 * D, D], F32,
                          kind="ExternalInput").ap()
    rg_b = nc.dram_tensor("rg_b", [NBLOCKS, D], F32, kind="ExternalInput").ap()
    ht_W = nc.dram_tensor("ht_W", [NBLOCKS, 2 * D, D], F32,
                          kind="ExternalInput").ap()
    ht_b = nc.dram_tensor("ht_b", [NBLOCKS, D], F32, kind="ExternalInput").ap()
    res2_W = nc.dram_tensor("res2_W", [D, 1], F32, kind="ExternalInput").ap()
    res2_b = nc.dram_tensor("res2_b", [1], F32, kind="ExternalInput").ap()

    xout = nc.dram_tensor("xout", [NL, D], F32, kind="ExternalOutput").ap()
    smout = nc.dram_tensor("smout", [1, NL], F32, kind="ExternalOutput").ap()
    lossout = nc.dram_tensor("lossout", [1, 1], F32,
                             kind="ExternalOutput").ap()

    with tile.TileContext(nc) as tc, ExitStack() as ctx:
        p_adjT = ctx.enter_context(tc.tile_pool(name="adjT", bufs=1))
        p_state = ctx.enter_context(tc.tile_pool(name="state", bufs=1))
        p_ch = ctx.enter_context(tc.tile_pool(name="ch", bufs=2))
        p_msg = ctx.enter_context(tc.tile_pool(name="msg", bufs=2))
        p_w = ctx.enter_context(tc.tile_pool(name="w", bufs=2))
        p_const = ctx.enter_context(tc.tile_pool(name="const", bufs=1))
        ps_tp = ctx.enter_context(
            tc.tile_pool(name="ps_tp", bufs=3, space="PSUM"))
        ps_mm = ctx.enter_context(
            tc.tile_pool(name="ps_mm", bufs=2, space="PSUM"))
        ps_g = ctx.enter_context(
            tc.tile_pool(name="ps_g", bufs=3, space="PSUM"))

        ident = p_const.tile([P, P], F32)
        make_identity(nc, ident[:])
        ident_r = p_const.tile([P, P], F32)
        nc.vector.tensor_copy(_r(ident_r[:]), ident[:])

        # persistent state, feature-major: hT[d, n]
        hT = p_state.tile([P, N], F32)
        # adjT_big[p, mt*N + n] = adj[n, mt*128 + p]
        adjT_big = p_adjT.tile([P, NT * N], F32)
        adjT_v = adjT_big.rearrange("p (m n) -> p m n", m=NT)

        # ---- biases / small constants ----
        bias_in = p_const.tile([P, NBLOCKS], F32)
        nc.sync.dma_start(bias_in[:], in_b.rearrange("b d -> d b"))
        bias_ug = p_const.tile([P, NBLOCKS], F32)
        nc.sync.dma_start(bias_ug[:], ug_b.rearrange("b d -> d b"))
        bias_rg = p_const.tile([P, NBLOCKS], F32)
        nc.sync.dma_start(bias_rg[:], rg_b.rearrange("b d -> d b"))
        bias_ht = p_const.tile([P, NBLOCKS], F32)
        nc.sync.dma_start(bias_ht[:], ht_b.rearrange("b d -> d b"))
        res2w_s = p_const.tile([P, 1], F32)
        nc.sync.dma_start(res2w_s[:], res2_W[:, :])
        res2w = p_const.tile([P, 1], F32)
        nc.vector.tensor_copy(_r(res2w[:]), res2w_s[:])
        res2b = p_const.tile([1, 1], F32)
        nc.sync.dma_start(res2b[:], res2_b.rearrange("(a b) -> a b", a=1))
        res_row = p_const.tile([1, NL], F32)
        nc.sync.dma_start(res_row[:], res.rearrange("(a n) -> a n", a=1))

        # gather indices, one per partition: idx_t[p, t] = idx[t*128 + p]
        nidx_t = p_const.tile([P, NL // P], I32)
        nc.sync.dma_start(nidx_t[:], nidx.rearrange("(t p) -> p t", p=P))
        lidx_t = p_const.tile([P, LL // P], I32)
        nc.sync.dma_start(lidx_t[:], lidx.rearrange("(t p) -> p t", p=P))
        text_t = p_const.tile([P, NL // P], I32)
        nc.sync.dma_start(text_t[:], text.rearrange("(t p) -> p t", p=P))

        # resmask row, precomputed at load time (independent of the GGNN)
        mask = p_const.tile([1, NL], I32)
        # ---- adjacency load + on-chip transpose ----
        # full-row staging: 8KB contiguous per partition line keeps the
        # HBM DMA near peak rate (2KB lines measured at only ~107 GB/s)
        with tc.tile_pool(name="stage", bufs=2) as p_stage:
            nidx_row = p_stage.tile([1, NL], I32, tag="adj")
            nc.sync.dma_start(nidx_row[:],
                              nidx.rearrange("(a n) -> a n", a=1))
            nc.vector.tensor_scalar(out=mask[:], in0=nidx_row[:], scalar1=2,
                                    scalar2=None,
                                    op0=mybir.AluOpType.is_equal)
            for nb in range(NT):
                st = p_stage.tile([P, N], F32, tag="adj")
                eng = nc.sync if nb % 2 == 0 else nc.scalar
                eng.dma_start(st[:], adj[nb * P:(nb + 1) * P, :])
                for q in range(4):
                    mt0 = q * 4
                    pt = ps_tp.tile([P, CH], F32, tag="tp")
                    for j in range(4):
                        nc.tensor.transpose(
                            pt[:, j * P:(j + 1) * P],
                            st[:, (q * 4 + j) * P:(q * 4 + j + 1) * P],
                            ident[:])
                    # strided scatter into adjT_big: 4 m-tiles, n-block nb
                    nc.vector.tensor_copy(
                        _r(adjT_v[:, mt0:mt0 + 4, nb * P:(nb + 1) * P]),
                        pt[:].rearrange("p (m n) -> p m n", m=4))
        p_row = ctx.enter_context(tc.tile_pool(name="row", bufs=3))

        # warm the ACT Exp/Ln tables so the softmax tail doesn't pay
        # the two ~1.3us ACT_TABLE_LOADs serially at the end
        warm = p_const.tile([1, 1], F32)
        nc.scalar.activation(warm[:], res2b[:],
                             mybir.ActivationFunctionType.Exp)
        nc.scalar.activation(warm[:], warm[:],
                             mybir.ActivationFunctionType.Ln)


        # ---- embeddings -> hT (initial x, feature-major) ----
        # node embedding tile = [tok_emb row, text scalar] (128 features),
        # assembled node-major then PE-transposed into hT
        for g in range(NL // P // 4):
            pt = ps_tp.tile([P, CH], F32, tag="tp")
            for j in range(4):
                t = g * 4 + j
                ge = p_ch.tile([P, D], F32, tag="z")
                nc.gpsimd.indirect_dma_start(
                    out=ge[:, 0:D - 1], out_offset=None, in_=tok_emb[:, :],
                    in_offset=IndirectOffsetOnAxis(ap=nidx_t[:, t:t + 1],
                                                   axis=0))
                nc.vector.tensor_copy(ge[:, D - 1:D], text_t[:, t:t + 1])
                nc.tensor.transpose(pt[:, j * P:(j + 1) * P], ge[:],
                                    ident[:])
            nc.vector.tensor_copy(_r(hT[:, g * CH:(g + 1) * CH]), pt[:])

        for g in range(LL // P // 4):
            pt = ps_tp.tile([P, CH], F32, tag="tp")
            for j in range(4):
                t = g * 4 + j
                ge1 = p_ch.tile([P, D], F32, tag="z")
                nc.gpsimd.indirect_dma_start(
                    out=ge1[:], out_offset=None, in_=tok_emb1[:, :],
                    in_offset=IndirectOffsetOnAxis(ap=lidx_t[:, t:t + 1],
                                                   axis=0))
                nc.tensor.transpose(pt[:, j * P:(j + 1) * P], ge1[:],
                                    ident[:])
            nc.vector.tensor_copy(_r(hT[:, NL + g * CH:NL + (g + 1) * CH]),
                                  pt[:])

        # ---- GGNN blocks ----
        # h_nat is double-buffered; its transpose groups are software-
        # pipelined: group c for the NEXT step is emitted right after this
        # step's chunk-c state update (lagged two chunks on PE), and the
        # final group is carried into the next step's first message-matmul
        # accumulation so PE never sits on the ACT/DVE update roundtrip.
        def new_hnat():
            h_nat = p_state.tile([P, N], F32, tag="h_nat", bufs=2,
                                 name="h_nat")
            return h_nat

        def make_tp(h_dst, c):
            def emit():
                pt = ps_tp.tile([P, CH], F32, tag="tp", name="pt_tp")
                for j in range(4):
                    nb = c * 4 + j
                    nc.tensor.transpose(_r(pt[:, j * P:(j + 1) * P]),
                                        _r(hT[:, nb * P:(nb + 1) * P]),
                                        _r(ident_r[:]))
                if COPIES_ON_ACT:
                    nc.scalar.copy(_r(h_dst[:, c * CH:(c + 1) * CH]), pt[:])
                else:
                    nc.vector.tensor_copy(_r(h_dst[:, c * CH:(c + 1) * CH]),
                                          pt[:])
            return emit

        carry_tp = None
        for blk in range(nblocks):
            ws = p_ch.tile([P, 2, D], F32, tag="t")
            nc.sync.dma_start(ws[:, 0, :], in_W[blk])
            w_in = p_w.tile([P, D], F32, tag="w_in")
            nc.vector.tensor_copy(_r(w_in[:]), ws[:, 0, :])
            ws = p_ch.tile([P, 2, D], F32, tag="t")
            nc.sync.dma_start(ws[:],
                              ug_W[blk].rearrange("(k p) d -> p k d", p=P))
            w_ug = p_w.tile([P, 2, D], F32, tag="w_ug")
            nc.vector.tensor_copy(_r(w_ug[:]), ws[:])
            ws = p_ch.tile([P, 2, D], F32, tag="t")
            nc.sync.dma_start(ws[:],
                              rg_W[blk].rearrange("(k p) d -> p k d", p=P))
            w_rg = p_w.tile([P, 2, D], F32, tag="w_rg")
            nc.vector.tensor_copy(_r(w_rg[:]), ws[:])
            ws = p_ch.tile([P, 2, D], F32, tag="t")
            nc.sync.dma_start(ws[:],
                              ht_W[blk].rearrange("(k p) d -> p k d", p=P))
            w_ht = p_w.tile([P, 2, D], F32, tag="w_ht")
            nc.vector.tensor_copy(_r(w_ht[:]), ws[:])

            # h = x @ in_W + in_b chunk-wise in place on hT, with the
            # transpose groups for step 0 pipelined one chunk behind
            h_cur = new_hnat()
            tp_q = []
            for c in range(NCH):
                pm = ps_g.tile([P, CH], F32, tag="g")
                nc.tensor.matmul(pm[:], _r(w_in[:]),
                                 _r(hT[:, c * CH:(c + 1) * CH]),
                                 start=True, stop=True)
                nc.vector.tensor_scalar(
                    out=_r(hT[:, c * CH:(c + 1) * CH]), in0=pm[:],
                    scalar1=bias_in[:, blk:blk + 1], scalar2=None,
                    op0=mybir.AluOpType.add)
                if tp_q:
                    tp_q.pop(0)()
                tp_q.append(make_tp(h_cur, c))
            carry_tp = tp_q.pop(0)

            for step in range(nsteps):
                last_step = step == nsteps - 1
                h_next = None if last_step else new_hnat()

                tails = []  # pending ht-candidate finishers, lag 1
                tps = []    # pending transpose groups for h_next, lag 2
                for c in range(NCH):
                    cs = slice(c * CH, (c + 1) * CH)
                    pmsg = ps_mm.tile([P, CH], F32, tag="m", name="pmsg")
                    pz = ps_g.tile([P, CH], F32, tag="g", name="pz")
                    pr = ps_g.tile([P, CH], F32, tag="g", name="pr")

                    # msgs first half
                    for mt in range(8):
                        nc.tensor.matmul(
                            pmsg[:], _r(h_cur[:, mt * P:(mt + 1) * P]),
                            _r(adjT_v[:, mt, c * CH:(c + 1) * CH]),
                            start=(mt == 0), stop=False)
                    # gate h-halves: no msgs dependency, keeps PE busy and
                    # shortens the post-copy critical path
                    nc.tensor.matmul(pz[:], _r(w_ug[:, 0, :]), _r(hT[:, cs]),
                                     start=True, stop=False)
                    nc.tensor.matmul(pr[:], _r(w_rg[:, 0, :]), _r(hT[:, cs]),
                                     start=True, stop=False)
                    if tails:
                        tails[0][0]()  # t_{c-1} rh-half
                    # msgs second half, with the carried transpose group
                    # (writes h_cur chunk 3 = m-tiles 12..15) at mt == 8
                    for mt in range(8, NT):
                        if mt == 8 and c == 0 and carry_tp is not None:
                            carry_tp()
                            carry_tp = None
                        nc.tensor.matmul(
                            pmsg[:], _r(h_cur[:, mt * P:(mt + 1) * P]),
                            _r(adjT_v[:, mt, c * CH:(c + 1) * CH]),
                            start=False, stop=(mt == NT - 1))
                    msgs_c = p_msg.tile([P, CH], F32, tag="msg",
                                        name="msgs_c")
                    nc.vector.tensor_copy(_r(msgs_c[:]), pmsg[:])

                    if tails:
                        tails.pop(0)[1]()  # t_{c-1} finish
                    if c >= 2 and tps:
                        tps.pop(0)()

                    # gate msgs-halves + activations
                    nc.tensor.matmul(pz[:], _r(w_ug[:, 1, :]), _r(msgs_c[:]),
                                     start=False, stop=True)
                    z_sb = p_ch.tile([P, CH], F32, tag="z")
                    nc.scalar.activation(z_sb[:], pz[:],
                                         mybir.ActivationFunctionType.Sigmoid,
                                         bias=bias_ug[:, blk:blk + 1])
                    nc.tensor.matmul(pr[:], _r(w_rg[:, 1, :]), _r(msgs_c[:]),
                                     start=False, stop=True)
                    r_c = p_ch.tile([P, CH], F32, tag="r", bufs=1)
                    nc.scalar.activation(r_c[:], pr[:],
                                         mybir.ActivationFunctionType.Sigmoid,
                                         bias=bias_rg[:, blk:blk + 1])
                    rh_c = p_ch.tile([P, CH], F32, tag="rh")
                    nc.vector.tensor_mul(_r(rh_c[:]), r_c[:], hT[:, cs])

                    # build this chunk's tail (two phases)
                    def make_tail(c, z_sb, msgs_c, rh_c):
                        cs = slice(c * CH, (c + 1) * CH)
                        pt2 = ps_g.tile([P, CH], F32, tag="g", name="pt2")

                        def phase_a():
                            nc.tensor.matmul(pt2[:], _r(w_ht[:, 0, :]),
                                             _r(rh_c[:]), start=True,
                                             stop=False)

                        def phase_b(msgs_c=msgs_c):
                            nc.tensor.matmul(pt2[:], _r(w_ht[:, 1, :]),
                                             _r(msgs_c[:]),
                                             start=False, stop=True)
                            t_sb = p_ch.tile([P, CH], F32, tag="t",
                                             name="t_sb")
                            nc.scalar.activation(
                                t_sb[:], pt2[:],
                                mybir.ActivationFunctionType.Tanh,
                                bias=bias_ht[:, blk:blk + 1])
                            d_c = p_ch.tile([P, CH], F32, tag="d", bufs=1,
                                            name="d_c")
                            nc.vector.tensor_sub(d_c[:], hT[:, cs], t_sb[:])
                            nc.vector.tensor_mul(d_c[:], z_sb[:], d_c[:])
                            nc.vector.tensor_add(_r(hT[:, cs]), d_c[:],
                                                 t_sb[:])
                        return phase_a, phase_b

                    tails.append(make_tail(c, z_sb, msgs_c, rh_c))
                    if h_next is not None:
                        tps.append(make_tp(h_next, c))

                # epilogue: cover the rh_3 roundtrip with tp(2), then finish
                if tps:
                    tps.pop(0)()
                pa, pb = tails.pop(0)
                pa()
                pb()
                carry_tp = tps.pop(0) if tps else None
                h_cur = h_next

        # ---- outputs ----
        # x output: node half of hT back to node-major
        xout_v = xout.rearrange("(a p) d -> p a d", p=P)
        for g in range(2):
            pt = ps_tp.tile([P, CH], F32, tag="tp")
            for j in range(4):
                nb = g * 4 + j
                nc.tensor.transpose(pt[:, j * P:(j + 1) * P],
                                    hT[:, nb * P:(nb + 1) * P], ident[:])
            xs = p_ch.tile([P, CH], F32, tag="z")
            nc.vector.tensor_copy(xs[:], pt[:])
            nc.sync.dma_start(xout_v[:, g * 4:(g + 1) * 4, :],
                              xs[:].rearrange("p (a d) -> p a d", a=4))

        # logits = x[:, :NL] @ res2_W + res2_b
        logits = p_row.tile([1, NL], F32, tag="row")
        for c in range(NL // CH):
            pl = ps_g.tile([1, CH], F32, tag="g")
            nc.tensor.matmul(pl[:], _r(res2w[:]),
                             _r(hT[:, c * CH:(c + 1) * CH]),
                             start=True, stop=True)
            nc.scalar.activation(logits[:, c * CH:(c + 1) * CH], pl[:],
                                 mybir.ActivationFunctionType.Identity,
                                 bias=res2b[:])

        # logits = where(resmask, logits, -1e9)
        masked = p_row.tile([1, NL], F32, tag="row")
        nc.vector.memset(masked[:], -1e9)
        nc.vector.copy_predicated(masked[:], mask[:], logits[:])

        # softmax along the row
        mx = p_const.tile([1, 1], F32)
        nc.vector.reduce_max(mx[:], masked[:], axis=mybir.AxisListType.X,
                             negate=True)
        ex = p_row.tile([1, NL], F32, tag="row")
        nc.scalar.activation(ex[:], masked[:],
                             mybir.ActivationFunctionType.Exp,
                             bias=mx[:], scale=1.0)
        sm_sum = p_const.tile([1, 1], F32)
        nc.vector.reduce_sum(sm_sum[:], ex[:], axis=mybir.AxisListType.X)
        nc.vector.reciprocal(sm_sum[:], sm_sum[:])
        smx = p_row.tile([1, NL], F32, tag="row")
        nc.vector.tensor_scalar_mul(smx[:], ex[:], sm_sum[:])
        nc.sync.dma_start(smout[:, :], smx[:])

        # loss = -(log(clip(softmax, 1e-10, 1)) * res).sum()
        cl = p_row.tile([1, NL], F32, tag="row")
        nc.vector.tensor_scalar(out=cl[:], in0=smx[:], scalar1=1e-10,
                                scalar2=1.0, op0=mybir.AluOpType.max,
                                op1=mybir.AluOpType.min)
        lnr = p_row.tile([1, NL], F32, tag="row")
        nc.scalar.activation(lnr[:], cl[:], mybir.ActivationFunctionType.Ln)
        prd = p_row.tile([1, NL], F32, tag="row")
        nc.vector.tensor_mul(prd[:], lnr[:], res_row[:])
        ls = p_const.tile([1, 1], F32)
        nc.vector.reduce_sum(ls[:], prd[:], axis=mybir.AxisListType.X)
        nc.vector.tensor_scalar_mul(ls[:], ls[:], -1.0)
        nc.sync.dma_start(lossout[:, :], ls[:])

    nc.compile()
    return nc


_NC = None
LAST_RESULT = None


def _get_nc():
    global _NC
    if _NC is None:
        _NC = build_nc()
    return _NC


def kernel(**inputs):
    nc = _get_nc()

    adj = np.ascontiguousarray(np.asarray(inputs["inputad"], np.float32))
    nidx = np.asarray(inputs["input_node"]).astype(np.int32)
    lidx = np.asarray(inputs["linenode"]).astype(np.int32)
    text = np.asarray(inputs["inputtext"]).astype(np.int32)
    res = np.asarray(inputs["res"]).astype(np.float32)
    shared = {
        "tok_emb": np.ascontiguousarray(np.asarray(inputs["tok_emb"], np.float32)),
        "tok_emb1": np.ascontiguousarray(np.asarray(inputs["tok_emb1"], np.float32)),
        "in_W": np.ascontiguousarray(np.asarray(inputs["in_W"], np.float32)),
        "in_b": np.ascontiguousarray(np.asarray(inputs["in_b"], np.float32)),
        "ug_W": np.ascontiguousarray(np.asarray(inputs["ug_W"], np.float32)),
        "ug_b": np.ascontiguousarray(np.asarray(inputs["ug_b"], np.float32)),
        "rg_W": np.ascontiguousarray(np.asarray(inputs["rg_W"], np.float32)),
        "rg_b": np.ascontiguousarray(np.asarray(inputs["rg_b"], np.float32)),
        "ht_W": np.ascontiguousarray(np.asarray(inputs["ht_W"], np.float32)),
        "ht_b": np.ascontiguousarray(np.asarray(inputs["ht_b"], np.float32)),
        "res2_W": np.ascontiguousarray(np.asarray(inputs["res2_W"], np.float32)),
        "res2_b": np.ascontiguousarray(np.asarray(inputs["res2_b"], np.float32)),
    }
    in_maps = []
    for b in range(N_CORES):
        in_maps.append({
            "adj": np.ascontiguousarray(adj[b]),
            "nidx": np.ascontiguousarray(nidx[b]),
            "lidx": np.ascontiguousarray(lidx[b]),
            "text": np.ascontiguousarray(text[b]),
            "res": np.ascontiguousarray(res[b]),
            **shared,
        })

    from concourse.bass_utils import run_bass_kernel_spmd
    global LAST_RESULT
    LAST_RESULT = run_bass_kernel_spmd(nc, in_maps,
                                       core_ids=list(range(N_CORES)))

    loss = np.zeros([B], np.float32)
    softmax = np.zeros([B, NL], np.float32)
    x = np.zeros([B, NL, D], np.float32)
    for b in range(N_CORES):
        r = LAST_RESULT.results[b]
        loss[b] = r["lossout"][0, 0]
        softmax[b] = r["smout"][0]
        x[b] = r["xout"]
    return loss, softmax, x
